# revision 20
# baseline (speedup 1.0000x reference)
"""Causal self-attention (4, 2048, 1024), 16 heads, on 8 trn2 NeuronCores.

Sharding: batch (4) x head-group (2 groups of 8 heads) -> 8 cores.
Each core computes, for its batch b and its 8 heads:
  qkv projection -> causal attention -> partial output projection
  partial_out = Y_heads @ w_proj[rows of those heads]
Host sums the two head-group partials per batch. No collectives.

Per-core kernel, software-pipelined over 512-wide t-chunks (tq = j):
  chunk tq: QKV projections for t in [512*tq, 512*tq+512)
            -> causal attention for the 512 queries of group j=tq
               (keys 0..512*(j+1) from KT/V history)
            -> output projection for those 512 t rows.
split_attn: the full (off-diagonal) key-blocks of chunk tq's attention are
emitted between the Q and K/V projections so their ACT-heavy exp work
overlaps projection PE work (attention grows with tq; this rebalances).

Attention inner loop is software-pipelined: PV matmuls for block l are
emitted after QK/exp of block l+1, so the in-order PE queue never stalls
on the exp->PV dependency. Causal masking (MASK_ON_PE selects variant)
is a DVE multiply on the exp output of diagonal blocks; the multiply
fits in the pipelined slack so it costs no PE time.

Softmax denominators ride the P@V matmul: each (ktile, head) weight
block is [V_h | ones] (even heads) or [ones | V_h] (odd heads), so the
[128,512] PSUM accumulator holds YT on one partition half and 64 copies
of the denominator row on the other, with all later elementwise ops
partition-base-aligned.

QK^T uses K=64 row-tiled matmul pairs (tiles (0,0)/(64,0) via
base_partition auto-derivation) which run concurrently in the PE array.
"""

import numpy as np

import concourse.bass as bass
import concourse.mybir as mybir
import concourse.tile as tile
from concourse import bacc

F32 = mybir.dt.float32
F32R = mybir.dt.float32r
BF16 = mybir.dt.bfloat16
FP16 = mybir.dt.float16

T = 2048   # sequence length
C = 1024   # embed dim
NP = 4     # head pairs per core (8 heads)
NKT = 16   # k-tiles of 128
EXPF = mybir.ActivationFunctionType.Exp
MASK_ON_PE = False  # causal mask: PE identity-matmul vs DVE multiply
SELMM_NORM = True   # denominators via 64x64 selector matmuls (no DMA bcast)


def build_nc(repeat=1):
    nc = bacc.Bacc(trn_type="TRN2", target_bir_lowering=False, debug=False,
                   num_devices=8)
    xT = nc.dram_tensor("xT", [C, T], FP16, kind="ExternalInput").ap()
    # wqkv cols: [q: 8 heads x 64 | k: 8 heads x 64 | v: 8 heads x 64]
    wqkv = nc.dram_tensor("wqkv", [C, 3 * 512], FP16, kind="ExternalInput").ap()
    wproj = nc.dram_tensor("wproj", [512, C], FP16, kind="ExternalInput").ap()
    # identity / causal-mask-bias pair for the diag-block mask matmul
    idm = nc.dram_tensor("idm", [128, 5, 128], FP16, kind="ExternalInput").ap()
    out = nc.dram_tensor("out", [T, C], FP16, kind="ExternalOutput").ap()

    with tile.TileContext(nc) as tc:
        build_body(tc, xT, wqkv, wproj, idm, out, repeat=repeat)
    nc.compile()
    return nc


def build_body(tc, xT, wqkv, wproj, idm, out, repeat=1):
    nc = tc.nc
    import contextlib
    ctx = contextlib.ExitStack()
    with ctx:
        persist = ctx.enter_context(tc.tile_pool(name="persist", bufs=1))
        xtp = ctx.enter_context(tc.tile_pool(name="xt_p", bufs=2))
        qslp = ctx.enter_context(tc.tile_pool(name="qsl_p", bufs=2))
        yslp = ctx.enter_context(tc.tile_pool(name="ysl_p", bufs=2))
        ep = ctx.enter_context(tc.tile_pool(name="e_p", bufs=6))
        rpp = ctx.enter_context(tc.tile_pool(name="rep_p", bufs=3))
        osp = ctx.enter_context(tc.tile_pool(name="osb_p", bufs=4))
        spp = ctx.enter_context(tc.tile_pool(name="spill_p", bufs=8))
        psp = ctx.enter_context(tc.tile_pool(name="ps_p", bufs=2,
                                             space="PSUM"))
        stp = ctx.enter_context(tc.tile_pool(name="st_p", bufs=2,
                                             space="PSUM"))
        ytp = ctx.enter_context(tc.tile_pool(name="yt_ps", bufs=2,
                                             space="PSUM"))

        kt_t = persist.tile([128, NP, T], FP16)      # KT pairs (d=128, t)
        # V+ones, fp16: per (ktile, head) block of 128 cols:
        # even heads [V_h | ones], odd heads [ones | V_h]
        vv_t = persist.tile([128, NKT, 8, 128], FP16)
        idm_t = persist.tile([128, 5, 128], FP16)    # [ident|mneg|mask01 x2|1/64]
        wv_t = persist.tile([128, 8, 512], FP16)     # V proj weights
        w_all = persist.tile([128, 8, 8, 128], FP16)  # QT/KT proj weights
        wp_t = persist.tile([128, NP, C], FP16)      # out proj weights

        CHUNKS = [(0, 512), (512, 512), (1024, 512), (1536, 512)]

        # chunk-0 x DMA first so the first Q matmuls unblock asap
        # (single-shot build only: under repeat>1 the pool slot is recycled
        # by later chunks, so each iteration must DMA its own chunk-0 x)
        xt0 = None
        if repeat == 1:
            xt0 = xtp.tile([128, 8, 512], FP16, tag="xt")
            for c in range(8):
                nc.sync.dma_start(out=xt0[:, c, :],
                                  in_=xT[128 * c:128 * (c + 1), 0:512])
        for m0 in range(8):
            nc.sync.dma_start(
                out=w_all[:, m0, :, :],
                in_=wqkv[:, 128 * m0:128 * (m0 + 1)].rearrange(
                    "(c p) n -> p c n", p=128))
        nc.sync.dma_start(out=idm_t[:], in_=idm[:])
        nc.sync.dma_start(
            out=wv_t[:],
            in_=wqkv[:, 1024:1536].rearrange("(c p) n -> p c n", p=128))
        # ones halves of the V+ones blocks (Pool engine; DVE is busier)
        vv5 = vv_t[:, :, :, :].rearrange(
            "p l (hp par) d -> p l hp par d", par=2)
        nc.gpsimd.memset(vv5[:, :, :, 0, 64:128], 1.0)
        nc.gpsimd.memset(vv5[:, :, :, 1, 0:64], 1.0)

        def attn_span(g, blocks, yA, yB, first, last, qsl, q0, W):
            # Software-pipelined: PV for block l is emitted after QK/exp of
            # block l+1 so the in-order PE queue never stalls on exp -> PV.
            kb0 = q0 // 128
            hA, hB = 2 * g, 2 * g + 1
            nb = len(blocks)
            pending = None

            def emit_pv(p):
                e, off, idx = p
                fl = dict(start=(first and idx == 0),
                          stop=(last and idx == nb - 1))
                l = blocks[idx]
                # head A: yA rows 0:64 = YT_A, 64:128 = sums_A
                nc.tensor.matmul(yA[:, off:W], vv_t[:, l, hA, :],
                                 e[:, 0, off:W], **fl)
                # head B: yB rows 0:64 = sums_B, 64:128 = YT_B
                nc.tensor.matmul(yB[:, off:W], vv_t[:, l, hB, :],
                                 e[:, 1, off:W], **fl)

            for idx, l in enumerate(blocks):
                off = 128 * l - q0 if l >= kb0 else 0
                diag = l >= kb0
                st = stp.tile([128, 2, 512], F32, tag="st")
                nc.tensor.matmul(
                    st[:, 0, off:W],
                    kt_t[0:64, g, 128 * l:128 * (l + 1)],
                    qsl[0:64, g, off:W],
                    start=True, stop=not (diag and MASK_ON_PE))
                nc.tensor.matmul(
                    st[:, 1, off:W],
                    kt_t[64:128, g, 128 * l:128 * (l + 1)],
                    qsl[64:128, g, off:W],
                    start=True, stop=not (diag and MASK_ON_PE))
                if diag and MASK_ON_PE:
                    # add -200 where q < k (strictly-lower of block)
                    nc.tensor.matmul(st[:, 0, off:off + 128],
                                     idm_t[:, 0, :], idm_t[:, 1, :],
                                     start=False, stop=True)
                    nc.tensor.matmul(st[:, 1, off:off + 128],
                                     idm_t[:, 0, :], idm_t[:, 1, :],
                                     start=False, stop=True)
                e = ep.tile([128, 2, 512], FP16, tag="e")
                nc.scalar.activation(e[:, :, off:W], st[:, :, off:W],
                                     EXPF, scale=0.125)
                if diag and not MASK_ON_PE:
                    # zero the strictly-lower triangle of the diag block
                    nc.vector.tensor_mul(e[:, :, off:off + 128],
                                         e[:, :, off:off + 128],
                                         idm_t[:, 2:4, :])
                if pending is not None:
                    emit_pv(pending)
                pending = (e, off, idx)
            emit_pv(pending)

        def x_and_qproj(tq):
            cq0, Wc = CHUNKS[tq]
            if tq == 0 and xt0 is not None:
                xt = xt0
            else:
                xt = xtp.tile([128, 8, 512], FP16, tag="xt")
                for c in range(8):
                    nc.sync.dma_start(
                        out=xt[:, c, 0:Wc],
                        in_=xT[128 * c:128 * (c + 1), cq0:cq0 + Wc])
            qsl = qslp.tile([128, NP, 512], FP16, tag="qsl")
            for m in range(4):  # QT pairs
                ps = psp.tile([128, 512], F32, tag="ps")
                for c in range(8):
                    nc.tensor.matmul(ps[:, 0:Wc], w_all[:, m, c, :],
                                     xt[:, c, 0:Wc],
                                     start=(c == 0), stop=(c == 7))
                nc.vector.tensor_copy(qsl[:, m, 0:Wc], ps[:, 0:Wc])
            return xt, qsl

        def kv_proj(tq, xt):
            cq0, Wc = CHUNKS[tq]
            for m in range(4, 8):  # KT pairs
                ps = psp.tile([128, 512], F32, tag="ps")
                for c in range(8):
                    nc.tensor.matmul(ps[:, 0:Wc], w_all[:, m, c, :],
                                     xt[:, c, 0:Wc],
                                     start=(c == 0), stop=(c == 7))
                nc.vector.tensor_copy(kt_t[:, m - 4, cq0:cq0 + Wc],
                                      ps[:, 0:Wc])
            for tt in range(Wc // 128):  # V for the t-tiles of this chunk
                ps = psp.tile([128, 512], F32, tag="ps")
                for c in range(8):
                    nc.tensor.matmul(ps[:],
                                     xt[:, c, 128 * tt:128 * (tt + 1)],
                                     wv_t[:, c, :],
                                     start=(c == 0), stop=(c == 7))
                l = cq0 // 128 + tt
                psr = ps[:].rearrange("p (hp par d) -> p hp par d",
                                      par=2, d=64)
                vv4 = vv_t[:, l, :, :].rearrange(
                    "p (hp par) d -> p hp par d", par=2)
                nc.vector.tensor_copy(vv4[:, :, 0, 0:64], psr[:, :, 0, :])
                nc.vector.tensor_copy(vv4[:, :, 1, 64:128], psr[:, :, 1, :])

        def fulls(tq, qsl):
            # all off-diagonal key-blocks of chunk tq (keys already exist)
            cq0, Wc = CHUNKS[tq]
            sp_list = []
            for g in range(NP):
                yAf = ytp.tile([128, 512], F32, tag="ytps")
                yBf = ytp.tile([128, 512], F32, tag="ytps")
                attn_span(g, list(range(0, cq0 // 128)), yAf, yBf,
                          True, True, qsl, cq0, Wc)
                sp = spp.tile([128, 2, 512], F32, tag="spill")
                nc.vector.tensor_copy(sp[:, 0, 0:Wc], yAf[:, 0:Wc])
                nc.vector.tensor_copy(sp[:, 1, 0:Wc], yBf[:, 0:Wc])
                sp_list.append(sp)
            return sp_list

        def diag_norm(tq, qsl, spill):
            cq0, W = CHUNKS[tq]
            nkb = (cq0 + W) // 128
            kb0 = cq0 // 128
            ysl = yslp.tile([128, NP, 512], FP16, tag="ysl")
            for g in range(NP):
                yA = ytp.tile([128, 512], F32, tag="ytps")
                yB = ytp.tile([128, 512], F32, tag="ytps")
                attn_span(g, list(range(kb0, nkb)), yA, yB,
                          True, True, qsl, cq0, W)
                # bounce PSUM accumulators to SBUF (+ spilled fulls), then
                # normalize by the softmax denominators
                ycp = rpp.tile([128, 2, 512],
                               FP16 if SELMM_NORM else F32, tag="ycp")
                if spill[g] is not None:
                    nc.vector.tensor_add(ycp[:, 0, 0:W], yA[:, 0:W],
                                         spill[g][:, 0, 0:W])
                    nc.vector.tensor_add(ycp[:, 1, 0:W], yB[:, 0:W],
                                         spill[g][:, 1, 0:W])
                else:
                    nc.vector.tensor_copy(ycp[:, 0, 0:W], yA[:, 0:W])
                    nc.vector.tensor_copy(ycp[:, 1, 0:W], yB[:, 0:W])
                if SELMM_NORM:
                    # Two concurrent 64x64 selector matmuls average the 64
                    # denominator copies onto opposite partition halves of
                    # one PSUM tile: rows 0:64 = d_A, 64:128 = d_B.
                    dsel = ytp.tile([128, 512], F32, tag="ytps")
                    nc.tensor.matmul(dsel[0:64, 0:W],
                                     idm_t[64:128, 4, 0:64],
                                     ycp[64:128, 0, 0:W],
                                     start=True, stop=True)
                    nc.tensor.matmul(dsel[64:128, 0:W],
                                     idm_t[0:64, 4, 0:64],
                                     ycp[0:64, 1, 0:W],
                                     start=True, stop=True)
                    rep = rpp.tile([128, 512], FP16, tag="rep")
                    with nc.allow_low_precision(
                            reason="softmax denom recip; fp16 ok"):
                        nc.vector.reciprocal(rep[:, 0:W], dsel[:, 0:W])
                    nc.vector.tensor_mul(ysl[0:64, g, 0:W],
                                         ycp[0:64, 0, 0:W],
                                         rep[0:64, 0:W])
                    nc.vector.tensor_mul(ysl[64:128, g, 0:W],
                                         ycp[64:128, 1, 0:W],
                                         rep[64:128, 0:W])
                else:
                    repA = rpp.tile([128, 512], FP16, tag="rep")
                    with nc.allow_low_precision(
                            reason="softmax denom recip; fp16 ok"):
                        nc.vector.reciprocal(repA[64:128, 0:W],
                                             ycp[64:128, 0, 0:W])
                    nc.sync.dma_start(out=repA[0:64, 0:W],
                                      in_=repA[64:128, 0:W])
                    nc.vector.tensor_mul(ysl[0:64, g, 0:W],
                                         ycp[0:64, 0, 0:W],
                                         repA[0:64, 0:W])
                    repB = rpp.tile([128, 512], FP16, tag="rep")
                    with nc.allow_low_precision(
                            reason="softmax denom recip; fp16 ok"):
                        nc.vector.reciprocal(repB[0:64, 0:W],
                                             ycp[0:64, 1, 0:W])
                    nc.sync.dma_start(out=repB[64:128, 0:W],
                                      in_=repB[0:64, 0:W])
                    nc.vector.tensor_mul(ysl[64:128, g, 0:W],
                                         ycp[64:128, 1, 0:W],
                                         repB[64:128, 0:W])

            return ysl

        def out_proj(tq, ysl):
            cq0, W = CHUNKS[tq]
            # ------- output projection for this chunk -------
            for tt2 in range(W // 128):
                for ec in range(2):
                    ps = ytp.tile([128, 512], F32, tag="ytps")
                    for g in range(NP):
                        nc.tensor.matmul(
                            ps[:], ysl[:, g, 128 * tt2:128 * (tt2 + 1)],
                            wp_t[:, g, 512 * ec:512 * (ec + 1)],
                            start=(g == 0), stop=(g == 3))
                    ob = osp.tile([128, 512], FP16, tag="ob")
                    if tq == 3 and ec == 0:
                        # tail: split evacuations across ScalarE and DVE so
                        # neither serializes the out-proj PSUM rotation
                        nc.scalar.copy(ob[:], ps[:])
                    else:
                        nc.vector.tensor_copy(ob[:], ps[:])
                    row = cq0 + 128 * tt2
                    nc.sync.dma_start(
                        out=out[row:row + 128, 512 * ec:512 * (ec + 1)],
                        in_=ob[:])

        def emit_chunks():
            # cross-slot pipeline: slot tq runs chunk tq's K/V projections,
            # chunk tq+1's x-DMA/Q-projection and ALL of chunk tq+1's
            # off-diagonal attention (their keys already exist), then chunk
            # tq's diagonal blocks, normalize and output projection.  This
            # levels the exp (ScalarE) load across slots and shrinks the
            # exposed tail after the last projection.
            xt, qsl = x_and_qproj(0)
            spill = [None] * NP
            for tq in range(4):
                kv_proj(tq, xt)
                if tq == 0:
                    # deferred prologue load (needed from out-projection j=0)
                    nc.sync.dma_start(
                        out=wp_t[:],
                        in_=wproj.rearrange("(g p) n -> p g n", p=128))
                ysl = diag_norm(tq, qsl, spill)
                nxt = None
                if tq < 3:
                    # next chunk's Q-proj + off-diagonal attention emitted
                    # between norm and out-proj: the in-order PE queue chews
                    # this while the norm DVE chains drain, so out-proj
                    # never exposes the norm latency
                    xt2, qsl2 = x_and_qproj(tq + 1)
                    nxt = (xt2, qsl2, fulls(tq + 1, qsl2))
                out_proj(tq, ysl)
                if nxt is not None:
                    xt, qsl, spill = nxt

        if repeat > 1:
            with tc.For_i(0, repeat, 1):
                emit_chunks()
        else:
            emit_chunks()

def make_core_inputs(x, w_attn, w_proj):
    """Host-side sharding: returns list of 8 input dicts."""
    x = np.asarray(x, dtype=np.float32)
    w_attn = np.asarray(w_attn, dtype=np.float32)
    w_proj = np.asarray(w_proj, dtype=np.float32)
    k = np.arange(128)
    ident = np.eye(128, dtype=np.float16)
    # mneg[p, q] = -200 where q < p (strictly lower triangle)
    mneg = np.where(k[None, :] < k[:, None], np.float16(-200),
                    np.float16(0))
    m01 = (k[None, :] >= k[:, None]).astype(np.float16)
    sel = np.full((128, 128), 1.0 / 64, dtype=np.float16)
    idm = np.ascontiguousarray(
        np.stack([ident, mneg, m01, m01, sel], axis=1)).astype(np.float16)
    in_maps = []
    for core in range(8):
        b, hg = divmod(core, 2)
        cs = 512 * hg
        wq = w_attn[:, cs:cs + 512]
        wk = w_attn[:, 1024 + cs:1024 + cs + 512]
        wv = w_attn[:, 2048 + cs:2048 + cs + 512]
        wqkv = np.ascontiguousarray(np.concatenate([wq, wk, wv], axis=1))
        in_maps.append({
            "xT": np.ascontiguousarray(x[b].T).astype(np.float16),
            "wqkv": wqkv.astype(np.float16),
            "wproj": np.ascontiguousarray(w_proj[cs:cs + 512, :]).astype(np.float16),
            "idm": idm,
        })
    return in_maps


_NC_CACHE = {}


def get_nc(repeat=1):
    key = f"nc{repeat}"
    if key not in _NC_CACHE:
        _NC_CACHE[key] = build_nc(repeat=repeat)
    return _NC_CACHE[key]


def kernel(x, w_attn, w_proj):
    from concourse.bass_utils import run_bass_kernel_spmd
    nc = get_nc()
    in_maps = make_core_inputs(x, w_attn, w_proj)
    res = run_bass_kernel_spmd(nc, in_maps, list(range(8)), trace=False)
    parts = [res.results[i]["out"].astype(np.float32) for i in range(8)]
    y = np.stack([parts[2 * b] + parts[2 * b + 1] for b in range(4)], axis=0)
    return y


# revision 24
# speedup vs baseline: 1.0151x; 1.0151x over previous
"""Causal self-attention (4, 2048, 1024), 16 heads, on 8 trn2 NeuronCores.

Sharding: batch (4) x head-group (2 groups of 8 heads) -> 8 cores.
Each core computes, for its batch b and its 8 heads:
  qkv projection -> causal attention -> partial output projection
  partial_out = Y_heads @ w_proj[rows of those heads]
Host sums the two head-group partials per batch. No collectives.

Cross-slot pipeline over 512-wide t-chunks: slot tq runs chunk tq's K/V
projections, chunk tq's diagonal attention blocks + softmax normalize,
then chunk tq+1's x-DMA/Q-projection and ALL of chunk tq+1's
off-diagonal attention (their keys already exist; results spill to SBUF
and are re-added at chunk tq+1's diagonal pass), and finally chunk tq's
output projection (so the normalize DVE chains hide under next-chunk PE
work).  This levels the exp (ScalarE) load across slots - attention work
grows with tq - and shrinks the exposed tail after the last projection.

Attention inner loop is software-pipelined: PV matmuls for block l are
emitted after QK/exp of block l+1, so the in-order PE queue never stalls
on the exp->PV dependency. Causal masking (MASK_ON_PE selects variant)
is a DVE multiply on the exp output of diagonal blocks; the multiply
fits in the pipelined slack so it costs no PE time.

Softmax denominators ride the P@V matmul: each (ktile, head) weight
block is [V_h | ones] (even heads) or [ones | V_h] (odd heads), so the
[128,512] PSUM accumulator holds YT on one partition half and 64 copies
of the denominator row on the other.  Per pair, two concurrent 64x64
selector matmuls (1/64 x the 64 denominator copies) place d_A and d_B
on opposite partition halves of one PSUM tile, so a single fp16
reciprocal + two fp16 multiplies normalize without any DMA partition
broadcast in the critical chain (SELMM_NORM selects variant).

QK^T uses K=64 row-tiled matmul pairs (tiles (0,0)/(64,0) via
base_partition auto-derivation) which run concurrently in the PE array.
"""

import numpy as np

import concourse.bass as bass
import concourse.mybir as mybir
import concourse.tile as tile
from concourse import bacc

F32 = mybir.dt.float32
F32R = mybir.dt.float32r
BF16 = mybir.dt.bfloat16
FP16 = mybir.dt.float16

T = 2048   # sequence length
C = 1024   # embed dim
NP = 4     # head pairs per core (8 heads)
NKT = 16   # k-tiles of 128
EXPF = mybir.ActivationFunctionType.Exp
MASK_ON_PE = False  # causal mask: PE identity-matmul vs DVE multiply
SELMM_NORM = True   # denominators via 64x64 selector matmuls (no DMA bcast)


def build_nc(repeat=1):
    nc = bacc.Bacc(trn_type="TRN2", target_bir_lowering=False, debug=False,
                   num_devices=8)
    xT = nc.dram_tensor("xT", [C, T], FP16, kind="ExternalInput").ap()
    # wqkv cols: [q: 8 heads x 64 | k: 8 heads x 64 | v: 8 heads x 64]
    wqkv = nc.dram_tensor("wqkv", [C, 3 * 512], FP16, kind="ExternalInput").ap()
    wproj = nc.dram_tensor("wproj", [512, C], FP16, kind="ExternalInput").ap()
    # identity / causal-mask-bias pair for the diag-block mask matmul
    idm = nc.dram_tensor("idm", [128, 5, 128], FP16, kind="ExternalInput").ap()
    out = nc.dram_tensor("out", [T, C], FP16, kind="ExternalOutput").ap()

    with tile.TileContext(nc) as tc:
        build_body(tc, xT, wqkv, wproj, idm, out, repeat=repeat)
    nc.compile()
    return nc


def build_body(tc, xT, wqkv, wproj, idm, out, repeat=1):
    nc = tc.nc
    import contextlib
    ctx = contextlib.ExitStack()
    with ctx:
        persist = ctx.enter_context(tc.tile_pool(name="persist", bufs=1))
        xtp = ctx.enter_context(tc.tile_pool(name="xt_p", bufs=2))
        qslp = ctx.enter_context(tc.tile_pool(name="qsl_p", bufs=2))
        yslp = ctx.enter_context(tc.tile_pool(name="ysl_p", bufs=2))
        ep = ctx.enter_context(tc.tile_pool(name="e_p", bufs=8))
        rpp = ctx.enter_context(tc.tile_pool(name="rep_p", bufs=3))
        osp = ctx.enter_context(tc.tile_pool(name="osb_p", bufs=4))
        spp = ctx.enter_context(tc.tile_pool(name="spill_p", bufs=8))
        psp = ctx.enter_context(tc.tile_pool(name="ps_p", bufs=2,
                                             space="PSUM"))
        stp = ctx.enter_context(tc.tile_pool(name="st_p", bufs=2,
                                             space="PSUM"))
        ytp = ctx.enter_context(tc.tile_pool(name="yt_ps", bufs=2,
                                             space="PSUM"))

        kt_t = persist.tile([128, NP, T], FP16)      # KT pairs (d=128, t)
        # V+ones, fp16: per (ktile, head) block of 128 cols:
        # even heads [V_h | ones], odd heads [ones | V_h]
        vv_t = persist.tile([128, NKT, 8, 128], FP16)
        idm_t = persist.tile([128, 5, 128], FP16)    # [ident|mneg|mask01 x2|1/64]
        wv_t = persist.tile([128, 8, 512], FP16)     # V proj weights
        w_all = persist.tile([128, 8, 8, 128], FP16)  # QT/KT proj weights
        wp_t = persist.tile([128, NP, C], FP16)      # out proj weights

        CHUNKS = [(0, 512), (512, 512), (1024, 512), (1536, 512)]

        # chunk-0 x DMA first so the first Q matmuls unblock asap
        # (single-shot build only: under repeat>1 the pool slot is recycled
        # by later chunks, so each iteration must DMA its own chunk-0 x)
        xt0 = None
        if repeat == 1:
            xt0 = xtp.tile([128, 8, 512], FP16, tag="xt")
            for c in range(8):
                nc.sync.dma_start(out=xt0[:, c, :],
                                  in_=xT[128 * c:128 * (c + 1), 0:512])
        for m0 in range(8):
            nc.sync.dma_start(
                out=w_all[:, m0, :, :],
                in_=wqkv[:, 128 * m0:128 * (m0 + 1)].rearrange(
                    "(c p) n -> p c n", p=128))
        nc.sync.dma_start(out=idm_t[:], in_=idm[:])
        nc.sync.dma_start(
            out=wv_t[:],
            in_=wqkv[:, 1024:1536].rearrange("(c p) n -> p c n", p=128))
        # ones halves of the V+ones blocks (Pool engine; DVE is busier)
        vv5 = vv_t[:, :, :, :].rearrange(
            "p l (hp par) d -> p l hp par d", par=2)
        nc.gpsimd.memset(vv5[:, :, :, 0, 64:128], 1.0)
        nc.gpsimd.memset(vv5[:, :, :, 1, 0:64], 1.0)

        def attn_span(g, blocks, yA, yB, first, last, qsl, q0, W):
            # Software-pipelined two blocks deep with paired flushes: the PE
            # stream is QK(2i) QK(2i+1) PV(2i-2) PV(2i-1) ..., so PVs never
            # stall on their exp (two QK pairs of cover) and QK<->PV tiling-
            # mode switches halve versus per-block alternation.
            kb0 = q0 // 128
            hA, hB = 2 * g, 2 * g + 1
            nb = len(blocks)
            pend = []

            def emit_pv(p):
                e, off, idx = p
                fl = dict(start=(first and idx == 0),
                          stop=(last and idx == nb - 1))
                l = blocks[idx]
                # head A: yA rows 0:64 = YT_A, 64:128 = sums_A
                nc.tensor.matmul(yA[:, off:W], vv_t[:, l, hA, :],
                                 e[:, 0, off:W], **fl)
                # head B: yB rows 0:64 = sums_B, 64:128 = YT_B
                nc.tensor.matmul(yB[:, off:W], vv_t[:, l, hB, :],
                                 e[:, 1, off:W], **fl)

            for idx, l in enumerate(blocks):
                off = 128 * l - q0 if l >= kb0 else 0
                diag = l >= kb0
                st = stp.tile([128, 2, 512], F32, tag="st")
                nc.tensor.matmul(
                    st[:, 0, off:W],
                    kt_t[0:64, g, 128 * l:128 * (l + 1)],
                    qsl[0:64, g, off:W],
                    start=True, stop=not (diag and MASK_ON_PE))
                nc.tensor.matmul(
                    st[:, 1, off:W],
                    kt_t[64:128, g, 128 * l:128 * (l + 1)],
                    qsl[64:128, g, off:W],
                    start=True, stop=not (diag and MASK_ON_PE))
                if diag and MASK_ON_PE:
                    # add -200 where q < k (strictly-lower of block)
                    nc.tensor.matmul(st[:, 0, off:off + 128],
                                     idm_t[:, 0, :], idm_t[:, 1, :],
                                     start=False, stop=True)
                    nc.tensor.matmul(st[:, 1, off:off + 128],
                                     idm_t[:, 0, :], idm_t[:, 1, :],
                                     start=False, stop=True)
                e = ep.tile([128, 2, 512], FP16, tag="e")
                nc.scalar.activation(e[:, :, off:W], st[:, :, off:W],
                                     EXPF, scale=0.125)
                if diag and not MASK_ON_PE:
                    # zero the strictly-lower triangle of the diag block
                    nc.vector.tensor_mul(e[:, :, off:off + 128],
                                         e[:, :, off:off + 128],
                                         idm_t[:, 2:4, :])
                pend.append((e, off, idx))
                if idx % 2 == 1:
                    while len(pend) > 2:
                        emit_pv(pend.pop(0))
            for p in pend:
                emit_pv(p)

        def x_and_qproj(tq):
            cq0, Wc = CHUNKS[tq]
            if tq == 0 and xt0 is not None:
                xt = xt0
            else:
                xt = xtp.tile([128, 8, 512], FP16, tag="xt")
                for c in range(8):
                    nc.sync.dma_start(
                        out=xt[:, c, 0:Wc],
                        in_=xT[128 * c:128 * (c + 1), cq0:cq0 + Wc])
            qsl = qslp.tile([128, NP, 512], FP16, tag="qsl")
            for m in range(4):  # QT pairs
                ps = psp.tile([128, 512], F32, tag="ps")
                for c in range(8):
                    nc.tensor.matmul(ps[:, 0:Wc], w_all[:, m, c, :],
                                     xt[:, c, 0:Wc],
                                     start=(c == 0), stop=(c == 7))
                nc.vector.tensor_copy(qsl[:, m, 0:Wc], ps[:, 0:Wc])
            return xt, qsl

        def kv_proj(tq, xt):
            cq0, Wc = CHUNKS[tq]
            for m in range(4, 8):  # KT pairs
                ps = psp.tile([128, 512], F32, tag="ps")
                for c in range(8):
                    nc.tensor.matmul(ps[:, 0:Wc], w_all[:, m, c, :],
                                     xt[:, c, 0:Wc],
                                     start=(c == 0), stop=(c == 7))
                nc.vector.tensor_copy(kt_t[:, m - 4, cq0:cq0 + Wc],
                                      ps[:, 0:Wc])
            for tt in range(Wc // 128):  # V for the t-tiles of this chunk
                ps = psp.tile([128, 512], F32, tag="ps")
                for c in range(8):
                    nc.tensor.matmul(ps[:],
                                     xt[:, c, 128 * tt:128 * (tt + 1)],
                                     wv_t[:, c, :],
                                     start=(c == 0), stop=(c == 7))
                l = cq0 // 128 + tt
                psr = ps[:].rearrange("p (hp par d) -> p hp par d",
                                      par=2, d=64)
                vv4 = vv_t[:, l, :, :].rearrange(
                    "p (hp par) d -> p hp par d", par=2)
                nc.vector.tensor_copy(vv4[:, :, 0, 0:64], psr[:, :, 0, :])
                nc.vector.tensor_copy(vv4[:, :, 1, 64:128], psr[:, :, 1, :])

        def fulls(tq, qsl):
            # all off-diagonal key-blocks of chunk tq (keys already exist)
            cq0, Wc = CHUNKS[tq]
            sp_list = []
            for g in range(NP):
                yAf = ytp.tile([128, 512], F32, tag="ytps")
                yBf = ytp.tile([128, 512], F32, tag="ytps")
                attn_span(g, list(range(0, cq0 // 128)), yAf, yBf,
                          True, True, qsl, cq0, Wc)
                sp = spp.tile([128, 2, 512], F32, tag="spill")
                nc.vector.tensor_copy(sp[:, 0, 0:Wc], yAf[:, 0:Wc])
                nc.vector.tensor_copy(sp[:, 1, 0:Wc], yBf[:, 0:Wc])
                sp_list.append(sp)
            return sp_list

        def diag_norm(tq, qsl, spill):
            cq0, W = CHUNKS[tq]
            nkb = (cq0 + W) // 128
            kb0 = cq0 // 128
            ysl = yslp.tile([128, NP, 512], FP16, tag="ysl")
            for g in range(NP):
                yA = ytp.tile([128, 512], F32, tag="ytps")
                yB = ytp.tile([128, 512], F32, tag="ytps")
                attn_span(g, list(range(kb0, nkb)), yA, yB,
                          True, True, qsl, cq0, W)
                # bounce PSUM accumulators to SBUF (+ spilled fulls), then
                # normalize by the softmax denominators
                ycp = rpp.tile([128, 2, 512],
                               FP16 if SELMM_NORM else F32, tag="ycp")
                if spill[g] is not None:
                    nc.vector.tensor_add(ycp[:, 0, 0:W], yA[:, 0:W],
                                         spill[g][:, 0, 0:W])
                    nc.vector.tensor_add(ycp[:, 1, 0:W], yB[:, 0:W],
                                         spill[g][:, 1, 0:W])
                else:
                    nc.vector.tensor_copy(ycp[:, 0, 0:W], yA[:, 0:W])
                    nc.vector.tensor_copy(ycp[:, 1, 0:W], yB[:, 0:W])
                if SELMM_NORM:
                    # Two concurrent 64x64 selector matmuls average the 64
                    # denominator copies onto opposite partition halves of
                    # one PSUM tile: rows 0:64 = d_A, 64:128 = d_B.
                    dsel = ytp.tile([128, 512], F32, tag="ytps")
                    nc.tensor.matmul(dsel[0:64, 0:W],
                                     idm_t[64:128, 4, 0:64],
                                     ycp[64:128, 0, 0:W],
                                     start=True, stop=True)
                    nc.tensor.matmul(dsel[64:128, 0:W],
                                     idm_t[0:64, 4, 0:64],
                                     ycp[0:64, 1, 0:W],
                                     start=True, stop=True)
                    rep = rpp.tile([128, 512], FP16, tag="rep")
                    with nc.allow_low_precision(
                            reason="softmax denom recip; fp16 ok"):
                        nc.vector.reciprocal(rep[:, 0:W], dsel[:, 0:W])
                    nc.vector.tensor_mul(ysl[0:64, g, 0:W],
                                         ycp[0:64, 0, 0:W],
                                         rep[0:64, 0:W])
                    nc.vector.tensor_mul(ysl[64:128, g, 0:W],
                                         ycp[64:128, 1, 0:W],
                                         rep[64:128, 0:W])
                else:
                    repA = rpp.tile([128, 512], FP16, tag="rep")
                    with nc.allow_low_precision(
                            reason="softmax denom recip; fp16 ok"):
                        nc.vector.reciprocal(repA[64:128, 0:W],
                                             ycp[64:128, 0, 0:W])
                    nc.sync.dma_start(out=repA[0:64, 0:W],
                                      in_=repA[64:128, 0:W])
                    nc.vector.tensor_mul(ysl[0:64, g, 0:W],
                                         ycp[0:64, 0, 0:W],
                                         repA[0:64, 0:W])
                    repB = rpp.tile([128, 512], FP16, tag="rep")
                    with nc.allow_low_precision(
                            reason="softmax denom recip; fp16 ok"):
                        nc.vector.reciprocal(repB[0:64, 0:W],
                                             ycp[0:64, 1, 0:W])
                    nc.sync.dma_start(out=repB[64:128, 0:W],
                                      in_=repB[0:64, 0:W])
                    nc.vector.tensor_mul(ysl[64:128, g, 0:W],
                                         ycp[64:128, 1, 0:W],
                                         repB[64:128, 0:W])

            return ysl

        def out_proj(tq, ysl):
            cq0, W = CHUNKS[tq]
            # ------- output projection for this chunk -------
            for tt2 in range(W // 128):
                for ec in range(2):
                    ps = ytp.tile([128, 512], F32, tag="ytps")
                    for g in range(NP):
                        nc.tensor.matmul(
                            ps[:], ysl[:, g, 128 * tt2:128 * (tt2 + 1)],
                            wp_t[:, g, 512 * ec:512 * (ec + 1)],
                            start=(g == 0), stop=(g == 3))
                    ob = osp.tile([128, 512], FP16, tag="ob")
                    if tq == 3 and ec == 0:
                        # tail: split evacuations across ScalarE and DVE so
                        # neither serializes the out-proj PSUM rotation
                        nc.scalar.copy(ob[:], ps[:])
                    else:
                        nc.vector.tensor_copy(ob[:], ps[:])
                    row = cq0 + 128 * tt2
                    nc.sync.dma_start(
                        out=out[row:row + 128, 512 * ec:512 * (ec + 1)],
                        in_=ob[:])

        def emit_chunks():
            # cross-slot pipeline: slot tq runs chunk tq's K/V projections,
            # chunk tq+1's x-DMA/Q-projection and ALL of chunk tq+1's
            # off-diagonal attention (their keys already exist), then chunk
            # tq's diagonal blocks, normalize and output projection.  This
            # levels the exp (ScalarE) load across slots and shrinks the
            # exposed tail after the last projection.
            xt, qsl = x_and_qproj(0)
            spill = [None] * NP
            for tq in range(4):
                kv_proj(tq, xt)
                if tq == 0:
                    # deferred prologue load (needed from out-projection j=0)
                    nc.sync.dma_start(
                        out=wp_t[:],
                        in_=wproj.rearrange("(g p) n -> p g n", p=128))
                ysl = diag_norm(tq, qsl, spill)
                nxt = None
                if tq < 3:
                    # next chunk's Q-proj + off-diagonal attention emitted
                    # between norm and out-proj: the in-order PE queue chews
                    # this while the norm DVE chains drain, so out-proj
                    # never exposes the norm latency
                    xt2, qsl2 = x_and_qproj(tq + 1)
                    nxt = (xt2, qsl2, fulls(tq + 1, qsl2))
                out_proj(tq, ysl)
                if nxt is not None:
                    xt, qsl, spill = nxt

        if repeat > 1:
            with tc.For_i(0, repeat, 1):
                emit_chunks()
        else:
            emit_chunks()

def make_core_inputs(x, w_attn, w_proj):
    """Host-side sharding: returns list of 8 input dicts."""
    x = np.asarray(x, dtype=np.float32)
    w_attn = np.asarray(w_attn, dtype=np.float32)
    w_proj = np.asarray(w_proj, dtype=np.float32)
    k = np.arange(128)
    ident = np.eye(128, dtype=np.float16)
    # mneg[p, q] = -200 where q < p (strictly lower triangle)
    mneg = np.where(k[None, :] < k[:, None], np.float16(-200),
                    np.float16(0))
    m01 = (k[None, :] >= k[:, None]).astype(np.float16)
    sel = np.full((128, 128), 1.0 / 64, dtype=np.float16)
    idm = np.ascontiguousarray(
        np.stack([ident, mneg, m01, m01, sel], axis=1)).astype(np.float16)
    in_maps = []
    for core in range(8):
        b, hg = divmod(core, 2)
        cs = 512 * hg
        wq = w_attn[:, cs:cs + 512]
        wk = w_attn[:, 1024 + cs:1024 + cs + 512]
        wv = w_attn[:, 2048 + cs:2048 + cs + 512]
        wqkv = np.ascontiguousarray(np.concatenate([wq, wk, wv], axis=1))
        in_maps.append({
            "xT": np.ascontiguousarray(x[b].T).astype(np.float16),
            "wqkv": wqkv.astype(np.float16),
            "wproj": np.ascontiguousarray(w_proj[cs:cs + 512, :]).astype(np.float16),
            "idm": idm,
        })
    return in_maps


_NC_CACHE = {}


def get_nc(repeat=1):
    key = f"nc{repeat}"
    if key not in _NC_CACHE:
        _NC_CACHE[key] = build_nc(repeat=repeat)
    return _NC_CACHE[key]


def kernel(x, w_attn, w_proj):
    from concourse.bass_utils import run_bass_kernel_spmd
    nc = get_nc()
    in_maps = make_core_inputs(x, w_attn, w_proj)
    res = run_bass_kernel_spmd(nc, in_maps, list(range(8)), trace=False)
    parts = [res.results[i]["out"].astype(np.float32) for i in range(8)]
    y = np.stack([parts[2 * b] + parts[2 * b + 1] for b in range(4)], axis=0)
    return y


# revision 26
# speedup vs baseline: 1.0392x; 1.0238x over previous
"""Causal self-attention (4, 2048, 1024), 16 heads, on 8 trn2 NeuronCores.

Sharding: batch (4) x head-group (2 groups of 8 heads) -> 8 cores.
Each core computes, for its batch b and its 8 heads:
  qkv projection -> causal attention -> partial output projection
  partial_out = Y_heads @ w_proj[rows of those heads]
Host sums the two head-group partials per batch. No collectives.

Cross-slot pipeline over 512-wide t-chunks: slot tq runs chunk tq's K/V
projections, chunk tq's diagonal attention blocks + softmax normalize,
then chunk tq+1's x-DMA/Q-projection and ALL of chunk tq+1's
off-diagonal attention (their keys already exist; results spill to SBUF
and are re-added at chunk tq+1's diagonal pass), and finally chunk tq's
output projection (so the normalize DVE chains hide under next-chunk PE
work).  This levels the exp (ScalarE) load across slots - attention work
grows with tq - and shrinks the exposed tail after the last projection.

Attention inner loop is software-pipelined two blocks deep with paired
flushes (PE stream: QK(2i) QK(2i+1) PV(2i-2) PV(2i-1) ...), so PVs never
stall on their exp and QK<->PV tiling-mode switches halve versus
per-block alternation. Causal masking (MASK_ON_PE selects variant)
is a DVE multiply on the exp output of diagonal blocks; the multiply
fits in the pipelined slack so it costs no PE time.

Softmax denominators ride the P@V matmul: each (ktile, head) weight
block is [V_h | ones] (even heads) or [ones | V_h] (odd heads), so the
[128,512] PSUM accumulator holds YT on one partition half and 64 copies
of the denominator row on the other.  Per pair, two concurrent 64x64
selector matmuls (1/64 x the 64 denominator copies) place d_A and d_B
on opposite partition halves of one PSUM tile, so a single fp16
reciprocal + two fp16 multiplies normalize without any DMA partition
broadcast in the critical chain (SELMM_NORM selects variant).

QK^T uses K=64 row-tiled matmul pairs (tiles (0,0)/(64,0) via
base_partition auto-derivation) which run concurrently in the PE array.
"""

import numpy as np

import concourse.bass as bass
import concourse.mybir as mybir
import concourse.tile as tile
from concourse import bacc

F32 = mybir.dt.float32
F32R = mybir.dt.float32r
BF16 = mybir.dt.bfloat16
FP16 = mybir.dt.float16

T = 2048   # sequence length
C = 1024   # embed dim
NP = 4     # head pairs per core (8 heads)
NKT = 16   # k-tiles of 128
EXPF = mybir.ActivationFunctionType.Exp
MASK_ON_PE = False  # causal mask: PE identity-matmul vs DVE multiply
SELMM_NORM = True   # denominators via 64x64 selector matmuls (no DMA bcast)


def build_nc(repeat=1):
    nc = bacc.Bacc(trn_type="TRN2", target_bir_lowering=False, debug=False,
                   num_devices=8)
    xT = nc.dram_tensor("xT", [C, T], FP16, kind="ExternalInput").ap()
    # host-pre-transposed weights: every DMA is per-partition contiguous
    wqkvr = nc.dram_tensor("wqkvr", [128, 8, 8, 128], FP16,
                           kind="ExternalInput").ap()   # [p, m, c, n] q|k
    wvr = nc.dram_tensor("wvr", [128, 8, 512], FP16,
                         kind="ExternalInput").ap()     # [p, c, n] v
    wpr = nc.dram_tensor("wpr", [128, 4, 1024], FP16,
                         kind="ExternalInput").ap()     # [p, g, n] proj
    # identity / causal-mask-bias pair for the diag-block mask matmul
    idm = nc.dram_tensor("idm", [128, 5, 128], FP16, kind="ExternalInput").ap()
    out = nc.dram_tensor("out", [T, C], FP16, kind="ExternalOutput").ap()

    with tile.TileContext(nc) as tc:
        build_body(tc, xT, wqkvr, wvr, wpr, idm, out, repeat=repeat)
    nc.compile()
    return nc


def build_body(tc, xT, wqkvr, wvr, wpr, idm, out, repeat=1):
    nc = tc.nc
    import contextlib
    ctx = contextlib.ExitStack()
    with ctx:
        persist = ctx.enter_context(tc.tile_pool(name="persist", bufs=1))
        xtp = ctx.enter_context(tc.tile_pool(name="xt_p", bufs=2))
        qslp = ctx.enter_context(tc.tile_pool(name="qsl_p", bufs=2))
        yslp = ctx.enter_context(tc.tile_pool(name="ysl_p", bufs=2))
        ep = ctx.enter_context(tc.tile_pool(name="e_p", bufs=8))
        rpp = ctx.enter_context(tc.tile_pool(name="rep_p", bufs=3))
        osp = ctx.enter_context(tc.tile_pool(name="osb_p", bufs=4))
        spp = ctx.enter_context(tc.tile_pool(name="spill_p", bufs=8))
        psp = ctx.enter_context(tc.tile_pool(name="ps_p", bufs=2,
                                             space="PSUM"))
        stp = ctx.enter_context(tc.tile_pool(name="st_p", bufs=2,
                                             space="PSUM"))
        ytp = ctx.enter_context(tc.tile_pool(name="yt_ps", bufs=2,
                                             space="PSUM"))

        kt_t = persist.tile([128, NP, T], FP16)      # KT pairs (d=128, t)
        # V+ones, fp16: per (ktile, head) block of 128 cols:
        # even heads [V_h | ones], odd heads [ones | V_h]
        vv_t = persist.tile([128, NKT, 8, 128], FP16)
        idm_t = persist.tile([128, 5, 128], FP16)    # [ident|mneg|mask01 x2|1/64]
        wv_t = persist.tile([128, 8, 512], FP16)     # V proj weights
        w_all = persist.tile([128, 8, 8, 128], FP16)  # QT/KT proj weights
        wp_t = persist.tile([128, NP, C], FP16)      # out proj weights

        CHUNKS = [(0, 512), (512, 512), (1024, 512), (1536, 512)]

        # chunk-0 x DMA first so the first Q matmuls unblock asap
        # (single-shot build only: under repeat>1 the pool slot is recycled
        # by later chunks, so each iteration must DMA its own chunk-0 x)
        xt0 = None
        if repeat == 1:
            xt0 = xtp.tile([128, 8, 512], FP16, tag="xt")
            for c in range(8):
                nc.sync.dma_start(out=xt0[:, c, :],
                                  in_=xT[128 * c:128 * (c + 1), 0:512])
        for m0 in range(8):
            nc.sync.dma_start(out=w_all[:, m0, :, :], in_=wqkvr[:, m0, :, :])
        nc.sync.dma_start(out=idm_t[:], in_=idm[:])
        nc.sync.dma_start(out=wv_t[:], in_=wvr[:])
        # ones halves of the V+ones blocks (Pool engine; DVE is busier)
        vv5 = vv_t[:, :, :, :].rearrange(
            "p l (hp par) d -> p l hp par d", par=2)
        nc.gpsimd.memset(vv5[:, :, :, 0, 64:128], 1.0)
        nc.gpsimd.memset(vv5[:, :, :, 1, 0:64], 1.0)

        def attn_span(g, blocks, yA, yB, first, last, qsl, q0, W):
            # Software-pipelined two blocks deep with paired flushes: the PE
            # stream is QK(2i) QK(2i+1) PV(2i-2) PV(2i-1) ..., so PVs never
            # stall on their exp (two QK pairs of cover) and QK<->PV tiling-
            # mode switches halve versus per-block alternation.
            kb0 = q0 // 128
            hA, hB = 2 * g, 2 * g + 1
            nb = len(blocks)
            pend = []

            def emit_pv(p):
                e, off, idx = p
                fl = dict(start=(first and idx == 0),
                          stop=(last and idx == nb - 1))
                l = blocks[idx]
                # head A: yA rows 0:64 = YT_A, 64:128 = sums_A
                nc.tensor.matmul(yA[:, off:W], vv_t[:, l, hA, :],
                                 e[:, 0, off:W], **fl)
                # head B: yB rows 0:64 = sums_B, 64:128 = YT_B
                nc.tensor.matmul(yB[:, off:W], vv_t[:, l, hB, :],
                                 e[:, 1, off:W], **fl)

            for idx, l in enumerate(blocks):
                off = 128 * l - q0 if l >= kb0 else 0
                diag = l >= kb0
                st = stp.tile([128, 2, 512], F32, tag="st")
                nc.tensor.matmul(
                    st[:, 0, off:W],
                    kt_t[0:64, g, 128 * l:128 * (l + 1)],
                    qsl[0:64, g, off:W],
                    start=True, stop=not (diag and MASK_ON_PE))
                nc.tensor.matmul(
                    st[:, 1, off:W],
                    kt_t[64:128, g, 128 * l:128 * (l + 1)],
                    qsl[64:128, g, off:W],
                    start=True, stop=not (diag and MASK_ON_PE))
                if diag and MASK_ON_PE:
                    # add -200 where q < k (strictly-lower of block)
                    nc.tensor.matmul(st[:, 0, off:off + 128],
                                     idm_t[:, 0, :], idm_t[:, 1, :],
                                     start=False, stop=True)
                    nc.tensor.matmul(st[:, 1, off:off + 128],
                                     idm_t[:, 0, :], idm_t[:, 1, :],
                                     start=False, stop=True)
                e = ep.tile([128, 2, 512], FP16, tag="e")
                nc.scalar.activation(e[:, :, off:W], st[:, :, off:W],
                                     EXPF, scale=0.125)
                if diag and not MASK_ON_PE:
                    # zero the strictly-lower triangle of the diag block
                    nc.vector.tensor_mul(e[:, :, off:off + 128],
                                         e[:, :, off:off + 128],
                                         idm_t[:, 2:4, :])
                pend.append((e, off, idx))
                if idx % 2 == 1:
                    while len(pend) > 2:
                        emit_pv(pend.pop(0))
            for p in pend:
                emit_pv(p)

        def x_and_qproj(tq):
            cq0, Wc = CHUNKS[tq]
            if tq == 0 and xt0 is not None:
                xt = xt0
            else:
                xt = xtp.tile([128, 8, 512], FP16, tag="xt")
                for c in range(8):
                    nc.sync.dma_start(
                        out=xt[:, c, 0:Wc],
                        in_=xT[128 * c:128 * (c + 1), cq0:cq0 + Wc])
            qsl = qslp.tile([128, NP, 512], FP16, tag="qsl")
            for m in range(4):  # QT pairs
                ps = psp.tile([128, 512], F32, tag="ps")
                for c in range(8):
                    nc.tensor.matmul(ps[:, 0:Wc], w_all[:, m, c, :],
                                     xt[:, c, 0:Wc],
                                     start=(c == 0), stop=(c == 7))
                nc.vector.tensor_copy(qsl[:, m, 0:Wc], ps[:, 0:Wc])
            return xt, qsl

        def kv_proj(tq, xt):
            cq0, Wc = CHUNKS[tq]
            for m in range(4, 8):  # KT pairs
                ps = psp.tile([128, 512], F32, tag="ps")
                for c in range(8):
                    nc.tensor.matmul(ps[:, 0:Wc], w_all[:, m, c, :],
                                     xt[:, c, 0:Wc],
                                     start=(c == 0), stop=(c == 7))
                nc.vector.tensor_copy(kt_t[:, m - 4, cq0:cq0 + Wc],
                                      ps[:, 0:Wc])
            for tt in range(Wc // 128):  # V for the t-tiles of this chunk
                ps = psp.tile([128, 512], F32, tag="ps")
                for c in range(8):
                    nc.tensor.matmul(ps[:],
                                     xt[:, c, 128 * tt:128 * (tt + 1)],
                                     wv_t[:, c, :],
                                     start=(c == 0), stop=(c == 7))
                l = cq0 // 128 + tt
                psr = ps[:].rearrange("p (hp par d) -> p hp par d",
                                      par=2, d=64)
                vv4 = vv_t[:, l, :, :].rearrange(
                    "p (hp par) d -> p hp par d", par=2)
                nc.vector.tensor_copy(vv4[:, :, 0, 0:64], psr[:, :, 0, :])
                nc.vector.tensor_copy(vv4[:, :, 1, 64:128], psr[:, :, 1, :])

        def fulls(tq, qsl):
            # all off-diagonal key-blocks of chunk tq (keys already exist)
            cq0, Wc = CHUNKS[tq]
            sp_list = []
            for g in range(NP):
                yAf = ytp.tile([128, 512], F32, tag="ytps")
                yBf = ytp.tile([128, 512], F32, tag="ytps")
                attn_span(g, list(range(0, cq0 // 128)), yAf, yBf,
                          True, True, qsl, cq0, Wc)
                sp = spp.tile([128, 2, 512], F32, tag="spill")
                nc.vector.tensor_copy(sp[:, 0, 0:Wc], yAf[:, 0:Wc])
                nc.vector.tensor_copy(sp[:, 1, 0:Wc], yBf[:, 0:Wc])
                sp_list.append(sp)
            return sp_list

        def diag_norm(tq, qsl, spill):
            cq0, W = CHUNKS[tq]
            nkb = (cq0 + W) // 128
            kb0 = cq0 // 128
            ysl = yslp.tile([128, NP, 512], FP16, tag="ysl")
            for g in range(NP):
                yA = ytp.tile([128, 512], F32, tag="ytps")
                yB = ytp.tile([128, 512], F32, tag="ytps")
                attn_span(g, list(range(kb0, nkb)), yA, yB,
                          True, True, qsl, cq0, W)
                # bounce PSUM accumulators to SBUF (+ spilled fulls), then
                # normalize by the softmax denominators
                ycp = rpp.tile([128, 2, 512],
                               FP16 if SELMM_NORM else F32, tag="ycp")
                if spill[g] is not None:
                    nc.vector.tensor_add(ycp[:, 0, 0:W], yA[:, 0:W],
                                         spill[g][:, 0, 0:W])
                    nc.vector.tensor_add(ycp[:, 1, 0:W], yB[:, 0:W],
                                         spill[g][:, 1, 0:W])
                else:
                    nc.vector.tensor_copy(ycp[:, 0, 0:W], yA[:, 0:W])
                    nc.vector.tensor_copy(ycp[:, 1, 0:W], yB[:, 0:W])
                if SELMM_NORM:
                    # Two concurrent 64x64 selector matmuls average the 64
                    # denominator copies onto opposite partition halves of
                    # one PSUM tile: rows 0:64 = d_A, 64:128 = d_B.
                    dsel = ytp.tile([128, 512], F32, tag="ytps")
                    nc.tensor.matmul(dsel[0:64, 0:W],
                                     idm_t[64:128, 4, 0:64],
                                     ycp[64:128, 0, 0:W],
                                     start=True, stop=True)
                    nc.tensor.matmul(dsel[64:128, 0:W],
                                     idm_t[0:64, 4, 0:64],
                                     ycp[0:64, 1, 0:W],
                                     start=True, stop=True)
                    rep = rpp.tile([128, 512], FP16, tag="rep")
                    with nc.allow_low_precision(
                            reason="softmax denom recip; fp16 ok"):
                        nc.vector.reciprocal(rep[:, 0:W], dsel[:, 0:W])
                    nc.vector.tensor_mul(ysl[0:64, g, 0:W],
                                         ycp[0:64, 0, 0:W],
                                         rep[0:64, 0:W])
                    nc.vector.tensor_mul(ysl[64:128, g, 0:W],
                                         ycp[64:128, 1, 0:W],
                                         rep[64:128, 0:W])
                else:
                    repA = rpp.tile([128, 512], FP16, tag="rep")
                    with nc.allow_low_precision(
                            reason="softmax denom recip; fp16 ok"):
                        nc.vector.reciprocal(repA[64:128, 0:W],
                                             ycp[64:128, 0, 0:W])
                    nc.sync.dma_start(out=repA[0:64, 0:W],
                                      in_=repA[64:128, 0:W])
                    nc.vector.tensor_mul(ysl[0:64, g, 0:W],
                                         ycp[0:64, 0, 0:W],
                                         repA[0:64, 0:W])
                    repB = rpp.tile([128, 512], FP16, tag="rep")
                    with nc.allow_low_precision(
                            reason="softmax denom recip; fp16 ok"):
                        nc.vector.reciprocal(repB[0:64, 0:W],
                                             ycp[0:64, 1, 0:W])
                    nc.sync.dma_start(out=repB[64:128, 0:W],
                                      in_=repB[0:64, 0:W])
                    nc.vector.tensor_mul(ysl[64:128, g, 0:W],
                                         ycp[64:128, 1, 0:W],
                                         repB[64:128, 0:W])

            return ysl

        def out_proj(tq, ysl):
            cq0, W = CHUNKS[tq]
            # ------- output projection for this chunk -------
            for tt2 in range(W // 128):
                for ec in range(2):
                    ps = ytp.tile([128, 512], F32, tag="ytps")
                    for g in range(NP):
                        nc.tensor.matmul(
                            ps[:], ysl[:, g, 128 * tt2:128 * (tt2 + 1)],
                            wp_t[:, g, 512 * ec:512 * (ec + 1)],
                            start=(g == 0), stop=(g == 3))
                    ob = osp.tile([128, 512], FP16, tag="ob")
                    if tq == 3 and ec == 0:
                        # tail: split evacuations across ScalarE and DVE so
                        # neither serializes the out-proj PSUM rotation
                        nc.scalar.copy(ob[:], ps[:])
                    else:
                        nc.vector.tensor_copy(ob[:], ps[:])
                    row = cq0 + 128 * tt2
                    nc.sync.dma_start(
                        out=out[row:row + 128, 512 * ec:512 * (ec + 1)],
                        in_=ob[:])

        def emit_chunks():
            # cross-slot pipeline: slot tq runs chunk tq's K/V projections,
            # chunk tq+1's x-DMA/Q-projection and ALL of chunk tq+1's
            # off-diagonal attention (their keys already exist), then chunk
            # tq's diagonal blocks, normalize and output projection.  This
            # levels the exp (ScalarE) load across slots and shrinks the
            # exposed tail after the last projection.
            xt, qsl = x_and_qproj(0)
            spill = [None] * NP
            for tq in range(4):
                kv_proj(tq, xt)
                if tq == 0:
                    # deferred prologue load (needed from out-projection j=0)
                    nc.sync.dma_start(out=wp_t[:], in_=wpr[:])
                ysl = diag_norm(tq, qsl, spill)
                nxt = None
                if tq < 3:
                    # next chunk's Q-proj + off-diagonal attention emitted
                    # between norm and out-proj: the in-order PE queue chews
                    # this while the norm DVE chains drain, so out-proj
                    # never exposes the norm latency
                    xt2, qsl2 = x_and_qproj(tq + 1)
                    nxt = (xt2, qsl2, fulls(tq + 1, qsl2))
                out_proj(tq, ysl)
                if nxt is not None:
                    xt, qsl, spill = nxt

        if repeat > 1:
            with tc.For_i(0, repeat, 1):
                emit_chunks()
        else:
            emit_chunks()

def make_core_inputs(x, w_attn, w_proj):
    """Host-side sharding: returns list of 8 input dicts."""
    x = np.asarray(x, dtype=np.float32)
    w_attn = np.asarray(w_attn, dtype=np.float32)
    w_proj = np.asarray(w_proj, dtype=np.float32)
    k = np.arange(128)
    ident = np.eye(128, dtype=np.float16)
    # mneg[p, q] = -200 where q < p (strictly lower triangle)
    mneg = np.where(k[None, :] < k[:, None], np.float16(-200),
                    np.float16(0))
    m01 = (k[None, :] >= k[:, None]).astype(np.float16)
    sel = np.full((128, 128), 1.0 / 64, dtype=np.float16)
    idm = np.ascontiguousarray(
        np.stack([ident, mneg, m01, m01, sel], axis=1)).astype(np.float16)
    in_maps = []
    for core in range(8):
        b, hg = divmod(core, 2)
        cs = 512 * hg
        wq = w_attn[:, cs:cs + 512]
        wk = w_attn[:, 1024 + cs:1024 + cs + 512]
        wv = w_attn[:, 2048 + cs:2048 + cs + 512]
        wqk = np.concatenate([wq, wk], axis=1)            # [1024, 1024]
        wqkvr = np.ascontiguousarray(
            wqk.reshape(8, 128, 8, 128).transpose(1, 2, 0, 3))
        wvr = np.ascontiguousarray(wv.reshape(8, 128, 512).transpose(1, 0, 2))
        wpr = np.ascontiguousarray(
            w_proj[cs:cs + 512, :].reshape(4, 128, 1024).transpose(1, 0, 2))
        in_maps.append({
            "xT": np.ascontiguousarray(x[b].T).astype(np.float16),
            "wqkvr": wqkvr.astype(np.float16),
            "wvr": wvr.astype(np.float16),
            "wpr": wpr.astype(np.float16),
            "idm": idm,
        })
    return in_maps


_NC_CACHE = {}


def get_nc(repeat=1):
    key = f"nc{repeat}"
    if key not in _NC_CACHE:
        _NC_CACHE[key] = build_nc(repeat=repeat)
    return _NC_CACHE[key]


def kernel(x, w_attn, w_proj):
    from concourse.bass_utils import run_bass_kernel_spmd
    nc = get_nc()
    in_maps = make_core_inputs(x, w_attn, w_proj)
    res = run_bass_kernel_spmd(nc, in_maps, list(range(8)), trace=False)
    parts = [res.results[i]["out"].astype(np.float32) for i in range(8)]
    y = np.stack([parts[2 * b] + parts[2 * b + 1] for b in range(4)], axis=0)
    return y


# revision 27
# speedup vs baseline: 1.1048x; 1.0630x over previous
"""Causal self-attention (4, 2048, 1024), 16 heads, on 8 trn2 NeuronCores.

Sharding: batch (4) x head-group (2 groups of 8 heads) -> 8 cores.
Each core computes, for its batch b and its 8 heads:
  qkv projection -> causal attention -> partial output projection
  partial_out = Y_heads @ w_proj[rows of those heads]
Host sums the two head-group partials per batch. No collectives.

Cross-slot pipeline over 512-wide t-chunks: slot tq runs chunk tq's K/V
projections, chunk tq's diagonal attention blocks + softmax normalize,
then chunk tq+1's x-DMA/Q-projection and ALL of chunk tq+1's
off-diagonal attention (their keys already exist; results spill to SBUF
and are re-added at chunk tq+1's diagonal pass), and finally chunk tq's
output projection (so the normalize DVE chains hide under next-chunk PE
work).  This levels the exp (ScalarE) load across slots - attention work
grows with tq - and shrinks the exposed tail after the last projection.

Attention inner loop is software-pipelined two blocks deep with paired
flushes (PE stream: QK(2i) QK(2i+1) PV(2i-2) PV(2i-1) ...), so PVs never
stall on their exp and QK<->PV tiling-mode switches halve versus
per-block alternation. Causal masking (MASK_ON_PE selects variant)
is a DVE multiply on the exp output of diagonal blocks; the multiply
fits in the pipelined slack so it costs no PE time.

Softmax denominators ride the P@V matmul: each (ktile, head) weight
block is [V_h | ones] (even heads) or [ones | V_h] (odd heads), so the
[128,512] PSUM accumulator holds YT on one partition half and 64 copies
of the denominator row on the other.  Per pair, two concurrent 64x64
selector matmuls (1/64 x the 64 denominator copies) place d_A and d_B
on opposite partition halves of one PSUM tile, so a single fp16
reciprocal + two fp16 multiplies normalize without any DMA partition
broadcast in the critical chain (SELMM_NORM selects variant).

QK^T uses K=64 row-tiled matmul pairs (tiles (0,0)/(64,0) via
base_partition auto-derivation) which run concurrently in the PE array.
"""

import numpy as np

import concourse.bass as bass
import concourse.mybir as mybir
import concourse.tile as tile
from concourse import bacc

F32 = mybir.dt.float32
F32R = mybir.dt.float32r
BF16 = mybir.dt.bfloat16
FP16 = mybir.dt.float16

T = 2048   # sequence length
C = 1024   # embed dim
NP = 4     # head pairs per core (8 heads)
NKT = 16   # k-tiles of 128
EXPF = mybir.ActivationFunctionType.Exp
MASK_ON_PE = False  # causal mask: PE identity-matmul vs DVE multiply
SELMM_NORM = True   # denominators via 64x64 selector matmuls (no DMA bcast)


def build_nc(repeat=1):
    nc = bacc.Bacc(trn_type="TRN2", target_bir_lowering=False, debug=False,
                   num_devices=8)
    xT = nc.dram_tensor("xT", [C, T], FP16, kind="ExternalInput").ap()
    # host-pre-transposed weights: every DMA is per-partition contiguous
    wqkvr = nc.dram_tensor("wqkvr", [128, 8, 8, 128], FP16,
                           kind="ExternalInput").ap()   # [p, m, c, n] q|k
    wvr = nc.dram_tensor("wvr", [128, 8, 512], FP16,
                         kind="ExternalInput").ap()     # [p, c, n] v
    wpr = nc.dram_tensor("wpr", [128, 4, 1024], FP16,
                         kind="ExternalInput").ap()     # [p, g, n] proj
    # identity / causal-mask-bias pair for the diag-block mask matmul
    idm = nc.dram_tensor("idm", [128, 5, 128], FP16, kind="ExternalInput").ap()
    out = nc.dram_tensor("out", [T, C], FP16, kind="ExternalOutput").ap()

    with tile.TileContext(nc) as tc:
        build_body(tc, xT, wqkvr, wvr, wpr, idm, out, repeat=repeat)
    nc.compile()
    return nc


def build_body(tc, xT, wqkvr, wvr, wpr, idm, out, repeat=1):
    nc = tc.nc
    import contextlib
    ctx = contextlib.ExitStack()
    with ctx:
        persist = ctx.enter_context(tc.tile_pool(name="persist", bufs=1))
        xtp = ctx.enter_context(tc.tile_pool(name="xt_p", bufs=2))
        qslp = ctx.enter_context(tc.tile_pool(name="qsl_p", bufs=2))
        yslp = ctx.enter_context(tc.tile_pool(name="ysl_p", bufs=2))
        ep = ctx.enter_context(tc.tile_pool(name="e_p", bufs=8))
        rpp = ctx.enter_context(tc.tile_pool(name="rep_p", bufs=3))
        osp = ctx.enter_context(tc.tile_pool(name="osb_p", bufs=4))
        spp = ctx.enter_context(tc.tile_pool(name="spill_p", bufs=8))
        psp = ctx.enter_context(tc.tile_pool(name="ps_p", bufs=2,
                                             space="PSUM"))
        stp = ctx.enter_context(tc.tile_pool(name="st_p", bufs=2,
                                             space="PSUM"))
        ytp = ctx.enter_context(tc.tile_pool(name="yt_ps", bufs=2,
                                             space="PSUM"))

        kt_t = persist.tile([128, NP, T], FP16)      # KT pairs (d=128, t)
        # V+ones, fp16: per (ktile, head) block of 128 cols:
        # even heads [V_h | ones], odd heads [ones | V_h]
        vv_t = persist.tile([128, NKT, 8, 128], FP16)
        idm_t = persist.tile([128, 5, 128], FP16)    # [ident|mneg|mask01 x2|1/64]
        wv_t = persist.tile([128, 8, 512], FP16)     # V proj weights
        w_all = persist.tile([128, 8, 8, 128], FP16)  # QT/KT proj weights
        wp_t = persist.tile([128, NP, C], FP16)      # out proj weights

        CHUNKS = [(0, 512), (512, 512), (1024, 512), (1536, 512)]

        # chunk-0 x DMA first so the first Q matmuls unblock asap
        # (single-shot build only: under repeat>1 the pool slot is recycled
        # by later chunks, so each iteration must DMA its own chunk-0 x)
        xt0 = None
        if repeat == 1:
            xt0 = xtp.tile([128, 8, 512], FP16, tag="xt")
            for c in range(8):
                nc.sync.dma_start(out=xt0[:, c, :],
                                  in_=xT[128 * c:128 * (c + 1), 0:512])
        for m0 in range(8):
            nc.sync.dma_start(out=w_all[:, m0, :, :], in_=wqkvr[:, m0, :, :])
        nc.sync.dma_start(out=idm_t[:], in_=idm[:])
        nc.sync.dma_start(out=wv_t[:], in_=wvr[:])
        nc.sync.dma_start(out=wp_t[:], in_=wpr[:])
        # ones halves of the V+ones blocks (Pool engine; DVE is busier)
        vv5 = vv_t[:, :, :, :].rearrange(
            "p l (hp par) d -> p l hp par d", par=2)
        nc.gpsimd.memset(vv5[:, :, :, 0, 64:128], 1.0)
        nc.gpsimd.memset(vv5[:, :, :, 1, 0:64], 1.0)

        def attn_span(g, blocks, yA, yB, first, last, qsl, q0, W):
            # Software-pipelined two blocks deep with paired flushes: the PE
            # stream is QK(2i) QK(2i+1) PV(2i-2) PV(2i-1) ..., so PVs never
            # stall on their exp (two QK pairs of cover) and QK<->PV tiling-
            # mode switches halve versus per-block alternation.
            kb0 = q0 // 128
            hA, hB = 2 * g, 2 * g + 1
            nb = len(blocks)
            pend = []

            def emit_pv(p):
                e, off, idx = p
                fl = dict(start=(first and idx == 0),
                          stop=(last and idx == nb - 1))
                l = blocks[idx]
                # head A: yA rows 0:64 = YT_A, 64:128 = sums_A
                nc.tensor.matmul(yA[:, off:W], vv_t[:, l, hA, :],
                                 e[:, 0, off:W], **fl)
                # head B: yB rows 0:64 = sums_B, 64:128 = YT_B
                nc.tensor.matmul(yB[:, off:W], vv_t[:, l, hB, :],
                                 e[:, 1, off:W], **fl)

            for idx, l in enumerate(blocks):
                off = 128 * l - q0 if l >= kb0 else 0
                diag = l >= kb0
                st = stp.tile([128, 2, 512], F32, tag="st")
                nc.tensor.matmul(
                    st[:, 0, off:W],
                    kt_t[0:64, g, 128 * l:128 * (l + 1)],
                    qsl[0:64, g, off:W],
                    start=True, stop=not (diag and MASK_ON_PE))
                nc.tensor.matmul(
                    st[:, 1, off:W],
                    kt_t[64:128, g, 128 * l:128 * (l + 1)],
                    qsl[64:128, g, off:W],
                    start=True, stop=not (diag and MASK_ON_PE))
                if diag and MASK_ON_PE:
                    # add -200 where q < k (strictly-lower of block)
                    nc.tensor.matmul(st[:, 0, off:off + 128],
                                     idm_t[:, 0, :], idm_t[:, 1, :],
                                     start=False, stop=True)
                    nc.tensor.matmul(st[:, 1, off:off + 128],
                                     idm_t[:, 0, :], idm_t[:, 1, :],
                                     start=False, stop=True)
                e = ep.tile([128, 2, 512], FP16, tag="e")
                nc.scalar.activation(e[:, :, off:W], st[:, :, off:W],
                                     EXPF, scale=0.125)
                if diag and not MASK_ON_PE:
                    # zero the strictly-lower triangle of the diag block
                    nc.vector.tensor_mul(e[:, :, off:off + 128],
                                         e[:, :, off:off + 128],
                                         idm_t[:, 2:4, :])
                pend.append((e, off, idx))
                if idx % 2 == 1:
                    while len(pend) > 2:
                        emit_pv(pend.pop(0))
            for p in pend:
                emit_pv(p)

        def x_and_qproj(tq):
            cq0, Wc = CHUNKS[tq]
            if tq == 0 and xt0 is not None:
                xt = xt0
            else:
                xt = xtp.tile([128, 8, 512], FP16, tag="xt")
                for c in range(8):
                    nc.sync.dma_start(
                        out=xt[:, c, 0:Wc],
                        in_=xT[128 * c:128 * (c + 1), cq0:cq0 + Wc])
            qsl = qslp.tile([128, NP, 512], FP16, tag="qsl")
            for m in range(4):  # QT pairs
                ps = psp.tile([128, 512], F32, tag="ps")
                for c in range(8):
                    nc.tensor.matmul(ps[:, 0:Wc], w_all[:, m, c, :],
                                     xt[:, c, 0:Wc],
                                     start=(c == 0), stop=(c == 7))
                nc.vector.tensor_copy(qsl[:, m, 0:Wc], ps[:, 0:Wc])
            return xt, qsl

        def kv_proj(tq, xt):
            cq0, Wc = CHUNKS[tq]
            for m in range(4, 8):  # KT pairs
                ps = psp.tile([128, 512], F32, tag="ps")
                for c in range(8):
                    nc.tensor.matmul(ps[:, 0:Wc], w_all[:, m, c, :],
                                     xt[:, c, 0:Wc],
                                     start=(c == 0), stop=(c == 7))
                nc.vector.tensor_copy(kt_t[:, m - 4, cq0:cq0 + Wc],
                                      ps[:, 0:Wc])
            for tt in range(Wc // 128):  # V for the t-tiles of this chunk
                ps = psp.tile([128, 512], F32, tag="ps")
                for c in range(8):
                    nc.tensor.matmul(ps[:],
                                     xt[:, c, 128 * tt:128 * (tt + 1)],
                                     wv_t[:, c, :],
                                     start=(c == 0), stop=(c == 7))
                l = cq0 // 128 + tt
                psr = ps[:].rearrange("p (hp par d) -> p hp par d",
                                      par=2, d=64)
                vv4 = vv_t[:, l, :, :].rearrange(
                    "p (hp par) d -> p hp par d", par=2)
                nc.vector.tensor_copy(vv4[:, :, 0, 0:64], psr[:, :, 0, :])
                nc.vector.tensor_copy(vv4[:, :, 1, 64:128], psr[:, :, 1, :])

        def fulls(tq, qsl):
            # all off-diagonal key-blocks of chunk tq (keys already exist)
            cq0, Wc = CHUNKS[tq]
            sp_list = []
            for g in range(NP):
                yAf = ytp.tile([128, 512], F32, tag="ytps")
                yBf = ytp.tile([128, 512], F32, tag="ytps")
                attn_span(g, list(range(0, cq0 // 128)), yAf, yBf,
                          True, True, qsl, cq0, Wc)
                sp = spp.tile([128, 2, 512], F32, tag="spill")
                nc.vector.tensor_copy(sp[:, 0, 0:Wc], yAf[:, 0:Wc])
                nc.vector.tensor_copy(sp[:, 1, 0:Wc], yBf[:, 0:Wc])
                sp_list.append(sp)
            return sp_list

        def diag_norm(tq, qsl, spill):
            cq0, W = CHUNKS[tq]
            nkb = (cq0 + W) // 128
            kb0 = cq0 // 128
            ysl = yslp.tile([128, NP, 512], FP16, tag="ysl")
            for g in range(NP):
                yA = ytp.tile([128, 512], F32, tag="ytps")
                yB = ytp.tile([128, 512], F32, tag="ytps")
                attn_span(g, list(range(kb0, nkb)), yA, yB,
                          True, True, qsl, cq0, W)
                # bounce PSUM accumulators to SBUF (+ spilled fulls), then
                # normalize by the softmax denominators
                ycp = rpp.tile([128, 2, 512],
                               FP16 if SELMM_NORM else F32, tag="ycp")
                if spill[g] is not None:
                    nc.vector.tensor_add(ycp[:, 0, 0:W], yA[:, 0:W],
                                         spill[g][:, 0, 0:W])
                    nc.vector.tensor_add(ycp[:, 1, 0:W], yB[:, 0:W],
                                         spill[g][:, 1, 0:W])
                else:
                    nc.vector.tensor_copy(ycp[:, 0, 0:W], yA[:, 0:W])
                    nc.vector.tensor_copy(ycp[:, 1, 0:W], yB[:, 0:W])
                if SELMM_NORM:
                    # Two concurrent 64x64 selector matmuls average the 64
                    # denominator copies onto opposite partition halves of
                    # one PSUM tile: rows 0:64 = d_A, 64:128 = d_B.
                    dsel = ytp.tile([128, 512], F32, tag="ytps")
                    nc.tensor.matmul(dsel[0:64, 0:W],
                                     idm_t[64:128, 4, 0:64],
                                     ycp[64:128, 0, 0:W],
                                     start=True, stop=True)
                    nc.tensor.matmul(dsel[64:128, 0:W],
                                     idm_t[0:64, 4, 0:64],
                                     ycp[0:64, 1, 0:W],
                                     start=True, stop=True)
                    rep = rpp.tile([128, 512], FP16, tag="rep")
                    with nc.allow_low_precision(
                            reason="softmax denom recip; fp16 ok"):
                        nc.vector.reciprocal(rep[:, 0:W], dsel[:, 0:W])
                    nc.vector.tensor_mul(ysl[0:64, g, 0:W],
                                         ycp[0:64, 0, 0:W],
                                         rep[0:64, 0:W])
                    nc.vector.tensor_mul(ysl[64:128, g, 0:W],
                                         ycp[64:128, 1, 0:W],
                                         rep[64:128, 0:W])
                else:
                    repA = rpp.tile([128, 512], FP16, tag="rep")
                    with nc.allow_low_precision(
                            reason="softmax denom recip; fp16 ok"):
                        nc.vector.reciprocal(repA[64:128, 0:W],
                                             ycp[64:128, 0, 0:W])
                    nc.sync.dma_start(out=repA[0:64, 0:W],
                                      in_=repA[64:128, 0:W])
                    nc.vector.tensor_mul(ysl[0:64, g, 0:W],
                                         ycp[0:64, 0, 0:W],
                                         repA[0:64, 0:W])
                    repB = rpp.tile([128, 512], FP16, tag="rep")
                    with nc.allow_low_precision(
                            reason="softmax denom recip; fp16 ok"):
                        nc.vector.reciprocal(repB[0:64, 0:W],
                                             ycp[0:64, 1, 0:W])
                    nc.sync.dma_start(out=repB[64:128, 0:W],
                                      in_=repB[0:64, 0:W])
                    nc.vector.tensor_mul(ysl[64:128, g, 0:W],
                                         ycp[64:128, 1, 0:W],
                                         repB[64:128, 0:W])

            return ysl

        def out_proj(tq, ysl):
            cq0, W = CHUNKS[tq]
            # ------- output projection for this chunk -------
            for tt2 in range(W // 128):
                for ec in range(2):
                    ps = ytp.tile([128, 512], F32, tag="ytps")
                    for g in range(NP):
                        nc.tensor.matmul(
                            ps[:], ysl[:, g, 128 * tt2:128 * (tt2 + 1)],
                            wp_t[:, g, 512 * ec:512 * (ec + 1)],
                            start=(g == 0), stop=(g == 3))
                    ob = osp.tile([128, 512], FP16, tag="ob")
                    if tq == 3 and ec == 0:
                        # tail: split evacuations across ScalarE and DVE so
                        # neither serializes the out-proj PSUM rotation
                        nc.scalar.copy(ob[:], ps[:])
                    else:
                        nc.vector.tensor_copy(ob[:], ps[:])
                    row = cq0 + 128 * tt2
                    nc.sync.dma_start(
                        out=out[row:row + 128, 512 * ec:512 * (ec + 1)],
                        in_=ob[:])

        def emit_chunks():
            # cross-slot pipeline: slot tq runs chunk tq's K/V projections,
            # chunk tq+1's x-DMA/Q-projection and ALL of chunk tq+1's
            # off-diagonal attention (their keys already exist), then chunk
            # tq's diagonal blocks, normalize and output projection.  This
            # levels the exp (ScalarE) load across slots and shrinks the
            # exposed tail after the last projection.
            xt, qsl = x_and_qproj(0)
            spill = [None] * NP
            for tq in range(4):
                kv_proj(tq, xt)
                ysl = diag_norm(tq, qsl, spill)
                nxt = None
                if tq < 3:
                    # next chunk's Q-proj + off-diagonal attention emitted
                    # between norm and out-proj: the in-order PE queue chews
                    # this while the norm DVE chains drain, so out-proj
                    # never exposes the norm latency
                    xt2, qsl2 = x_and_qproj(tq + 1)
                    nxt = (xt2, qsl2, fulls(tq + 1, qsl2))
                out_proj(tq, ysl)
                if nxt is not None:
                    xt, qsl, spill = nxt

        if repeat > 1:
            with tc.For_i(0, repeat, 1):
                emit_chunks()
        else:
            emit_chunks()

def make_core_inputs(x, w_attn, w_proj):
    """Host-side sharding: returns list of 8 input dicts."""
    x = np.asarray(x, dtype=np.float32)
    w_attn = np.asarray(w_attn, dtype=np.float32)
    w_proj = np.asarray(w_proj, dtype=np.float32)
    k = np.arange(128)
    ident = np.eye(128, dtype=np.float16)
    # mneg[p, q] = -200 where q < p (strictly lower triangle)
    mneg = np.where(k[None, :] < k[:, None], np.float16(-200),
                    np.float16(0))
    m01 = (k[None, :] >= k[:, None]).astype(np.float16)
    sel = np.full((128, 128), 1.0 / 64, dtype=np.float16)
    idm = np.ascontiguousarray(
        np.stack([ident, mneg, m01, m01, sel], axis=1)).astype(np.float16)
    in_maps = []
    for core in range(8):
        b, hg = divmod(core, 2)
        cs = 512 * hg
        wq = w_attn[:, cs:cs + 512]
        wk = w_attn[:, 1024 + cs:1024 + cs + 512]
        wv = w_attn[:, 2048 + cs:2048 + cs + 512]
        wqk = np.concatenate([wq, wk], axis=1)            # [1024, 1024]
        wqkvr = np.ascontiguousarray(
            wqk.reshape(8, 128, 8, 128).transpose(1, 2, 0, 3))
        wvr = np.ascontiguousarray(wv.reshape(8, 128, 512).transpose(1, 0, 2))
        wpr = np.ascontiguousarray(
            w_proj[cs:cs + 512, :].reshape(4, 128, 1024).transpose(1, 0, 2))
        in_maps.append({
            "xT": np.ascontiguousarray(x[b].T).astype(np.float16),
            "wqkvr": wqkvr.astype(np.float16),
            "wvr": wvr.astype(np.float16),
            "wpr": wpr.astype(np.float16),
            "idm": idm,
        })
    return in_maps


_NC_CACHE = {}


def get_nc(repeat=1):
    key = f"nc{repeat}"
    if key not in _NC_CACHE:
        _NC_CACHE[key] = build_nc(repeat=repeat)
    return _NC_CACHE[key]


def kernel(x, w_attn, w_proj):
    from concourse.bass_utils import run_bass_kernel_spmd
    nc = get_nc()
    in_maps = make_core_inputs(x, w_attn, w_proj)
    res = run_bass_kernel_spmd(nc, in_maps, list(range(8)), trace=False)
    parts = [res.results[i]["out"].astype(np.float32) for i in range(8)]
    y = np.stack([parts[2 * b] + parts[2 * b + 1] for b in range(4)], axis=0)
    return y
